# revision 19
# baseline (speedup 1.0000x reference)
"""Balanced BCE loss with per-sample dynamic top-k negative mining on 8 TRN2 cores.

Math: for each sample the reference computes
    pos_count = sum(gt*mask), neg_raw = sum((1-gt)*mask)
    neg_count = min(neg_raw, 3*pos_count), k = int(neg_count)
    loss = BCE(pred, gt);  pos_loss = sum(loss*positive)
    neg_topk = sum of k largest loss*negative values
    per_sample = (pos_loss + neg_topk) / (pos_count + neg_count + eps); mean over N.

Every negative position has loss > 0 (p is bounded away from {0,1}), so
whenever neg_raw <= 3*pos_count the top-k sum equals the FULL sum of negative
losses and the per-sample loss reduces to three streaming scalars:
    A = sum(gt*mask)   M = sum(mask)   (B = M - A)
    CD = sum_masked ln(p if gt else 1-p)     (= pos_loss + neg_sum, negated)
If a sample ever violates neg_raw <= 3*pos_count the host recomputes it
exactly (numpy).

Device mapping: data-parallel over N, 2 samples/core.  Each [640,640] sample
is a [128, 3200] view (12800B contiguous per partition), streamed in free-dim
chunks (small first chunk so compute starts early, small last chunk so the
post-DMA dependency tail is short).  Per chunk:
  - ScalarE: lp = Ln(p); l1p = Ln(1-p) (activation scale/bias); m16 = bf16
    "cast" of mask via Ln((e-1)*x+1) (exact 0->0, 1->1, keeps one activation
    table) whose accum_out is the per-chunk M column.
  - VectorE: gm = gt*mask (f32 in, bf16 out); u = m16*l1p (2x bf16) giving
    ln(1-p) on every masked position and 0 elsewhere; then
    copy_predicated(u, gm, lp) overwrites masked positives with ln(p).
    u is now the complete per-element masked log-loss (= C+D contribution)
    in THREE VectorE passes total -- no nm/t1/t2 intermediates.
  - TensorE: two [1,400] PSUM accumulators per sample via a stationary
    ones[128,1] bf16 vector: accCD sums u, accA sums gm (pos_count).
  - The final 400-wide chunk bypasses the PE: VectorE tensor_reduces write
    its A/CD columns directly, keeping the last-chunk chain short.
GpSimd only does the ones[] memset: its software tensor ops contend with
VectorE for SBUF and slow DVE ~4x (measured), so no compute goes there.
Input DMAs are triggered from SP in pred,mask,gt order (ScalarE starts on
pred before gt lands); ALL output DMA triggers are emitted after the last
input trigger so SP's in-order stream never head-of-line blocks input DMAs
behind compute (measured: a mid-stream blocked output trigger stalled the
remaining input stream by ~7us).  Host sums the per-partition/per-chunk
partials in float64; M and A are exact integers (0/1 tensors, f32
accumulators), so B = M - A is exact and the fast/fallback decision is
robust.  Only ln values are rounded to bf16 (~2^-9 relative, averaging out
over ~100k summed elements per sample).
"""

import os
import sys

# defensive: if a previous process left a NeuronCore wedged, ask NRT to
# reset cores at init (read before first jax/NRT touch; harmless otherwise)
os.environ.setdefault("NEURON_RT_RESET_CORES", "1")

if "/opt/trn_rl_repo" not in sys.path:
    sys.path.insert(0, "/opt/trn_rl_repo")

import numpy as np

N, H, W = 16, 640, 640
NEG_RATIO = 3.0
EPS = 1e-8
N_CORES = 8
S = N // N_CORES          # samples per core
P = 128
FREE = H * W // P         # 3200
CHUNK_PLANS = ((400, 1200, 1600), (1600, 1200, 400))
MM = 400                  # matmul sub-chunk (PSUM bank: <=512 f32)
# stats columns: one M column per chunk, plus [A, CD] for the PE-bypass
# final chunk of the last sample
NCOLS = tuple(len(p) + (2 if s == S - 1 else 0)
              for s, p in enumerate(CHUNK_PLANS))
OUTW = 4 * MM + 8     # [cd0|a0|cd1|a1|stats]

_STATE = {}


def _build():
    import concourse.bass as bass
    import concourse.tile as tile
    from concourse import bacc, mybir

    f32 = mybir.dt.float32
    bf16 = mybir.dt.bfloat16
    Alu = mybir.AluOpType
    Act = mybir.ActivationFunctionType
    Ax = mybir.AxisListType

    nc = bacc.Bacc("TRN2", target_bir_lowering=False, debug=False,
                   num_devices=N_CORES)
    pred_d = nc.dram_tensor("pred", [S, H, W], f32, kind="ExternalInput").ap()
    gt_d = nc.dram_tensor("gt", [S, H, W], f32, kind="ExternalInput").ap()
    mask_d = nc.dram_tensor("mask", [S, H, W], f32, kind="ExternalInput").ap()
    # single contiguous single-partition output: [cd0|a0|cd1|a1|stats(8)]
    out_d = nc.dram_tensor("out", [1, OUTW], f32, kind="ExternalOutput").ap()

    views = {"p": pred_d, "g": gt_d, "m": mask_d}
    chunks = []           # (s, c, CH, sl)
    for s in range(S):
        off = 0
        for c, CH in enumerate(CHUNK_PLANS[s]):
            chunks.append((s, c, CH, slice(off, off + CH)))
            off += CH

    with tile.TileContext(nc) as tc:
        with tc.tile_pool(name="cst", bufs=1) as cst, \
             tc.tile_pool(name="inp", bufs=4) as inp, \
             tc.tile_pool(name="mid", bufs=3) as mid, \
             tc.tile_pool(name="res", bufs=1) as res, \
             tc.tile_pool(name="ps", bufs=2, space="PSUM") as psp:
            ones = cst.tile([P, 1], bf16)
            nc.gpsimd.memset(ones[:], 1.0)
            onesf = cst.tile([P, 1], f32)
            nc.gpsimd.memset(onesf[:], 1.0)
            stats0 = res.tile([P, NCOLS[0]], f32)
            stats1 = res.tile([P, NCOLS[1]], f32)
            stats = (stats0, stats1)
            out_sb = res.tile([1, OUTW], f32)

            # input tiles + staggered triggers: each chunk's pred is
            # requested one chunk ahead of its mask/gt so ScalarE (the
            # pacing engine) never waits on pred arrival
            tiles = {}
            for i, (s, c, CH, sl) in enumerate(chunks):
                for t in ("p", "m", "g"):
                    tiles[(t, i)] = inp.tile([P, CH], f32, tag=t,
                                             name=f"t{t}_{s}_{c}")
            seq = [("p", 0)]
            for i in range(len(chunks)):
                if i + 1 < len(chunks):
                    seq.append(("p", i + 1))
                seq.append(("m", i))
                seq.append(("g", i))
            for t, i in seq:
                s, c, CH, sl = chunks[i]
                view = views[t][s].rearrange("(p a) w -> p (a w)", p=P)
                nc.sync.dma_start(tiles[(t, i)][:], view[:, sl])

            accs = {}
            pending_copies = []
            for s in range(S):
                accs[s] = (psp.tile([1, MM], f32, tag="accCD",
                                    name=f"accCD_{s}"),
                           psp.tile([1, MM], f32, tag="accA",
                                    name=f"accA_{s}"))
            accST = psp.tile([1, 8], f32, tag="accST", bufs=1)

            step = {s: 0 for s in range(S)}
            for i, (s, c, CH, sl) in enumerate(chunks):
                accCD, accA = accs[s]
                CHUNKS = CHUNK_PLANS[s]
                bypass_pe = s == S - 1 and c == len(CHUNKS) - 1
                nsteps = sum(CHUNKS[cc] // MM for cc in range(len(CHUNKS))
                             if not (s == S - 1 and cc == len(CHUNKS) - 1))
                tp, tm, tg = (tiles[("p", i)], tiles[("m", i)],
                              tiles[("g", i)])
                st = stats[s]

                if bypass_pe:
                    # this sample's accumulators closed at the previous
                    # chunk's matmuls; copy them out before the tail chain
                    nc.vector.tensor_copy(out_sb[:, 2 * MM:3 * MM], accCD[:])
                    nc.vector.tensor_copy(out_sb[:, 3 * MM:4 * MM], accA[:])

                # lp/l1p first: pred lands first (staggered triggers)
                lp = mid.tile([P, CH], bf16, tag="lp", name=f"lp_{s}_{c}")
                nc.scalar.activation(lp[:], tp[:], Act.Ln)
                l1p = mid.tile([P, CH], bf16, tag="l1p", name=f"l1p_{s}_{c}")
                nc.scalar.activation(l1p[:], tp[:], Act.Ln,
                                     bias=1.0, scale=-1.0)
                # bf16 "cast" of the 0/1 mask as ln((e-1)*x + 1), exactly
                # 0->0, 1->1; accum gives the M column for free
                m16 = mid.tile([P, CH], bf16, tag="m16", name=f"m16_{s}_{c}")
                nc.scalar.activation(m16[:], tm[:], Act.Ln,
                                     bias=1.0, scale=float(np.e - 1.0),
                                     accum_out=st[:, c:c + 1])
                gm = mid.tile([P, CH], bf16, tag="gm", name=f"gm_{s}_{c}")
                nc.vector.tensor_tensor(gm[:], tg[:], tm[:], Alu.mult)
                # overwrite l1p in place with lp wherever gt==1: predicate
                # is the raw gt tile bitcast to int32 (1.0f != 0)
                nc.vector.copy_predicated(l1p[:],
                                          tg[:].bitcast(mybir.dt.int32),
                                          lp[:])
                # u = masked chosen-log: ln(p) on masked positives,
                # ln(1-p) on masked negatives, 0 elsewhere
                u = mid.tile([P, CH], bf16, tag="u", name=f"u_{s}_{c}")
                nc.vector.tensor_tensor(u[:], m16[:], l1p[:], Alu.mult)

                if bypass_pe:
                    jA = len(CHUNKS)
                    nc.vector.tensor_reduce(st[:, jA:jA + 1], gm[:],
                                            Ax.X, Alu.add)
                    nc.vector.tensor_reduce(st[:, jA + 1:jA + 2], u[:],
                                            Ax.X, Alu.add)
                else:
                    for mblk in range(CH // MM):
                        nc.tensor.matmul(accA[:], ones[:],
                                         gm[:, bass.ts(mblk, MM)],
                                         start=step[s] == 0,
                                         stop=step[s] == nsteps - 1)
                        nc.tensor.matmul(accCD[:], ones[:],
                                         u[:, bass.ts(mblk, MM)],
                                         start=step[s] == 0,
                                         stop=step[s] == nsteps - 1)
                        step[s] += 1
                # previous sample's PSUM->SBUF copies go after this chunk's
                # VectorE work so VectorE never stalls on the PE close
                if pending_copies:
                    for dst_sb, acc_ap in pending_copies:
                        nc.vector.tensor_copy(dst_sb, acc_ap)
                    pending_copies = []
                if s == 0 and c == len(CHUNKS) - 1:
                    accCD_, accA_ = accs[0]
                    pending_copies = [(out_sb[:, 0:MM], accCD_[:]),
                                      (out_sb[:, MM:2 * MM], accA_[:])]

            # partition-reduce the per-partition stats columns with two
            # tiny f32 matmuls so the whole result leaves in ONE contiguous
            # single-partition DMA (a [128,k] f32 output DMA is 128 tiny
            # strided descriptors -- measured ~2-3us of endgame)
            nc.tensor.matmul(accST[:, 0:NCOLS[0]], onesf[:], stats0[:],
                             start=True, stop=True)
            nc.tensor.matmul(accST[:, NCOLS[0]:NCOLS[0] + NCOLS[1]],
                             onesf[:], stats1[:], start=True, stop=True)
            nc.vector.tensor_copy(out_sb[:, 4 * MM:4 * MM + 8], accST[:])
            nc.sync.dma_start(out_d[:], out_sb[:])
    nc.compile()
    return nc


def _get_nc():
    if "nc" not in _STATE:
        _STATE["nc"] = _build()
    return _STATE["nc"]


def _host_topk_fallback(p, g, m):
    """Exact per-sample reference semantics in numpy (rare path)."""
    p = p.astype(np.float32)
    positive = g * m
    negative = (1.0 - g) * m
    pos_count = positive.sum(dtype=np.float64)
    neg_count = min(negative.sum(dtype=np.float64), pos_count * NEG_RATIO)
    log_p = np.maximum(np.log(p), -100.0)
    log_1mp = np.maximum(np.log1p(-p), -100.0)
    loss = -(g * log_p + (1.0 - g) * log_1mp)
    pos_loss_sum = (loss * positive).sum(dtype=np.float64)
    neg_loss = (loss * negative).ravel()
    k = int(neg_count)
    if k > 0:
        top = np.partition(neg_loss, len(neg_loss) - k)[len(neg_loss) - k:]
        neg_topk = top.sum(dtype=np.float64)
    else:
        neg_topk = 0.0
    return (pos_loss_sum + neg_topk) / (pos_count + neg_count + EPS)


def _combine(results, p, g, m):
    losses = []
    for c in range(N_CORES):
        out = results[c]["out"].astype(np.float64).ravel()  # [OUTW]
        stc = out[4 * MM:]
        for s in range(S):
            nch = len(CHUNK_PLANS[s])
            base = 0 if s == 0 else NCOLS[0]
            M = stc[base:base + nch].sum()
            CD = out[2 * MM * s:2 * MM * s + MM].sum()
            A = out[2 * MM * s + MM:2 * MM * s + 2 * MM].sum()
            if s == S - 1:
                A += stc[base + nch]
                CD += stc[base + nch + 1]
            pos_count = round(A)
            neg_raw = round(M - A)
            if neg_raw <= pos_count * NEG_RATIO:
                # top-k covers every (strictly positive) negative loss
                losses.append((-CD) / (pos_count + neg_raw + EPS))
            else:
                i = c * S + s
                losses.append(_host_topk_fallback(p[i], g[i], m[i]))
    return np.float32(np.mean(losses))


def _in_maps(p, g, m):
    return [
        {"pred": p[c * S:(c + 1) * S],
         "gt": g[c * S:(c + 1) * S],
         "mask": m[c * S:(c + 1) * S]}
        for c in range(N_CORES)
    ]


def kernel(pred, gt, mask):
    from concourse import bass_utils

    p = np.ascontiguousarray(pred[:, 0], dtype=np.float32)   # [N,H,W]
    g = np.ascontiguousarray(gt, dtype=np.float32)
    m = np.ascontiguousarray(mask, dtype=np.float32)

    nc = _get_nc()
    in_maps = _in_maps(p, g, m)
    try:
        res = bass_utils.run_bass_kernel_spmd(nc, in_maps,
                                              core_ids=list(range(N_CORES)))
    except Exception:
        # one retry: transient device wedge from a prior process
        res = bass_utils.run_bass_kernel_spmd(nc, in_maps,
                                              core_ids=list(range(N_CORES)))
    return _combine(res.results, p, g, m)


# revision 21
# speedup vs baseline: 1.0641x; 1.0641x over previous
"""Balanced BCE loss with per-sample dynamic top-k negative mining on 8 TRN2 cores.

Math: for each sample the reference computes
    pos_count = sum(gt*mask), neg_raw = sum((1-gt)*mask)
    neg_count = min(neg_raw, 3*pos_count), k = int(neg_count)
    loss = BCE(pred, gt);  pos_loss = sum(loss*positive)
    neg_topk = sum of k largest loss*negative values
    per_sample = (pos_loss + neg_topk) / (pos_count + neg_count + eps); mean over N.

Every negative position has loss > 0 (p is bounded away from {0,1}), so
whenever neg_raw <= 3*pos_count the top-k sum equals the FULL sum of negative
losses and the per-sample loss reduces to three streaming scalars:
    A = sum(gt*mask)   M = sum(mask)   (B = M - A)
    CD = sum_masked ln(p if gt else 1-p)     (= pos_loss + neg_sum, negated)
If a sample ever violates neg_raw <= 3*pos_count the host recomputes it
exactly (numpy).

Device mapping: data-parallel over N, 2 samples/core.  Each [640,640] sample
is a [128, 3200] view (12800B contiguous per partition), streamed in free-dim
chunks (small first chunk so compute starts early, small last chunk so the
post-DMA dependency tail is short).  Per chunk:
  - ScalarE: lp = Ln(p); l1p = Ln(1-p) (activation scale/bias); m16 = bf16
    "cast" of mask via Ln((e-1)*x+1) (exact 0->0, 1->1, keeps one activation
    table) whose accum_out is the per-chunk M column.
  - VectorE: gm = gt*mask (f32 in, bf16 out); u = m16*l1p (2x bf16) giving
    ln(1-p) on every masked position and 0 elsewhere; then
    copy_predicated(u, gm, lp) overwrites masked positives with ln(p).
    u is now the complete per-element masked log-loss (= C+D contribution)
    in THREE VectorE passes total -- no nm/t1/t2 intermediates.
  - TensorE: two [1,400] PSUM accumulators per sample via a stationary
    ones[128,1] bf16 vector: accCD sums u, accA sums gm (pos_count).
  - The final 400-wide chunk bypasses the PE: VectorE tensor_reduces write
    its A/CD columns directly, keeping the last-chunk chain short.
GpSimd only does the ones[] memset: its software tensor ops contend with
VectorE for SBUF and slow DVE ~4x (measured), so no compute goes there.
Input DMAs are triggered from SP in pred,mask,gt order (ScalarE starts on
pred before gt lands); ALL output DMA triggers are emitted after the last
input trigger so SP's in-order stream never head-of-line blocks input DMAs
behind compute (measured: a mid-stream blocked output trigger stalled the
remaining input stream by ~7us).  Host sums the per-partition/per-chunk
partials in float64; M and A are exact integers (0/1 tensors, f32
accumulators), so B = M - A is exact and the fast/fallback decision is
robust.  Only ln values are rounded to bf16 (~2^-9 relative, averaging out
over ~100k summed elements per sample).
"""

import os
import sys

# defensive: if a previous process left a NeuronCore wedged, ask NRT to
# reset cores at init (read before first jax/NRT touch; harmless otherwise)
os.environ.setdefault("NEURON_RT_RESET_CORES", "1")

if "/opt/trn_rl_repo" not in sys.path:
    sys.path.insert(0, "/opt/trn_rl_repo")

import numpy as np

N, H, W = 16, 640, 640
NEG_RATIO = 3.0
EPS = 1e-8
N_CORES = 8
S = N // N_CORES          # samples per core
P = 128
FREE = H * W // P         # 3200
CHUNK_PLANS = ((800, 1600, 800), (800, 1600, 800))
MM = 400                  # matmul sub-chunk (PSUM bank: <=512 f32)
# stats columns: one M column per chunk, plus [A, CD] for the PE-bypass
# final chunk of the last sample
NCOLS = tuple(len(p) for p in CHUNK_PLANS)
NC01 = 3 + 3          # total stats columns (M per chunk)
OUTW = 4 * MM + NC01  # [cd0|a0|cd1|a1|stats]

_STATE = {}


def _build():
    import concourse.bass as bass
    import concourse.tile as tile
    from concourse import bacc, mybir

    f32 = mybir.dt.float32
    bf16 = mybir.dt.bfloat16
    Alu = mybir.AluOpType
    Act = mybir.ActivationFunctionType
    Ax = mybir.AxisListType

    nc = bacc.Bacc("TRN2", target_bir_lowering=False, debug=False,
                   num_devices=N_CORES)
    pred_d = nc.dram_tensor("pred", [S, H, W], f32, kind="ExternalInput").ap()
    gt_d = nc.dram_tensor("gt", [S, H, W], f32, kind="ExternalInput").ap()
    mask_d = nc.dram_tensor("mask", [S, H, W], f32, kind="ExternalInput").ap()
    # single contiguous single-partition output: [cd0|a0|cd1|a1|stats(8)]
    out_d = nc.dram_tensor("out", [1, OUTW], f32, kind="ExternalOutput").ap()

    views = {"p": pred_d, "g": gt_d, "m": mask_d}
    chunks = []           # (s, c, CH, sl)
    for s in range(S):
        off = 0
        for c, CH in enumerate(CHUNK_PLANS[s]):
            chunks.append((s, c, CH, slice(off, off + CH)))
            off += CH

    with tile.TileContext(nc) as tc:
        with tc.tile_pool(name="cst", bufs=1) as cst, \
             tc.tile_pool(name="inp", bufs=6) as inp, \
             tc.tile_pool(name="mid", bufs=3) as mid, \
             tc.tile_pool(name="res", bufs=1) as res, \
             tc.tile_pool(name="ps", bufs=2, space="PSUM") as psp:
            ones = cst.tile([P, 1], bf16)
            nc.gpsimd.memset(ones[:], 1.0)
            onesf = cst.tile([P, 1], f32)
            nc.gpsimd.memset(onesf[:], 1.0)
            stats0 = res.tile([P, NCOLS[0]], f32)
            stats1 = res.tile([P, NCOLS[1]], f32)
            stats = (stats0, stats1)
            out_sb = res.tile([1, OUTW], f32)

            # input tiles + staggered triggers: each chunk's pred is
            # requested one chunk ahead of its mask/gt so ScalarE (the
            # pacing engine) never waits on pred arrival
            tiles = {}
            for i, (s, c, CH, sl) in enumerate(chunks):
                for t in ("p", "m", "g"):
                    tiles[(t, i)] = inp.tile([P, CH], f32, tag=t,
                                             name=f"t{t}_{s}_{c}")
            seq = []
            for i in range(len(chunks)):
                seq.append(("p", i))
                seq.append(("m", i))
                seq.append(("g", i))
            for t, i in seq:
                s, c, CH, sl = chunks[i]
                view = views[t][s].rearrange("(p a) w -> p (a w)", p=P)
                nc.sync.dma_start(tiles[(t, i)][:], view[:, sl])

            accs = {}
            pending_copies = []
            for s in range(S):
                accs[s] = (psp.tile([1, MM], f32, tag="accCD",
                                    name=f"accCD_{s}"),
                           psp.tile([1, MM], f32, tag="accA",
                                    name=f"accA_{s}"))
            accST = psp.tile([1, NC01], f32, tag="accST", bufs=1)

            step = {s: 0 for s in range(S)}
            for i, (s, c, CH, sl) in enumerate(chunks):
                accCD, accA = accs[s]
                CHUNKS = CHUNK_PLANS[s]
                bypass_pe = False
                nsteps = sum(CHUNKS[cc] // MM for cc in range(len(CHUNKS)))
                tp, tm, tg = (tiles[("p", i)], tiles[("m", i)],
                              tiles[("g", i)])
                st = stats[s]

                # lp/l1p first: pred is the first DMA of the chunk
                lp = mid.tile([P, CH], bf16, tag="lp", name=f"lp_{s}_{c}")
                nc.scalar.activation(lp[:], tp[:], Act.Ln)
                l1p = mid.tile([P, CH], bf16, tag="l1p", name=f"l1p_{s}_{c}")
                nc.scalar.activation(l1p[:], tp[:], Act.Ln,
                                     bias=1.0, scale=-1.0)
                # bf16 "cast" of the 0/1 mask as ln((e-1)*x + 1), exactly
                # 0->0, 1->1; accum gives the M column for free
                m16 = mid.tile([P, CH], bf16, tag="m16", name=f"m16_{s}_{c}")
                nc.scalar.activation(m16[:], tm[:], Act.Ln,
                                     bias=1.0, scale=float(np.e - 1.0),
                                     accum_out=st[:, c:c + 1])
                gm = mid.tile([P, CH], bf16, tag="gm", name=f"gm_{s}_{c}")
                nc.vector.tensor_tensor(gm[:], tg[:], tm[:], Alu.mult)
                # overwrite l1p in place with lp wherever gt==1: predicate
                # is the raw gt tile bitcast to int32 (1.0f != 0)
                nc.vector.copy_predicated(l1p[:],
                                          tg[:].bitcast(mybir.dt.int32),
                                          lp[:])
                # u = masked chosen-log: ln(p) on masked positives,
                # ln(1-p) on masked negatives, 0 elsewhere
                u = mid.tile([P, CH], bf16, tag="u", name=f"u_{s}_{c}")
                nc.vector.tensor_tensor(u[:], m16[:], l1p[:], Alu.mult)

                if True:
                    for mblk in range(CH // MM):
                        nc.tensor.matmul(accA[:], ones[:],
                                         gm[:, bass.ts(mblk, MM)],
                                         start=step[s] == 0,
                                         stop=step[s] == nsteps - 1)
                        nc.tensor.matmul(accCD[:], ones[:],
                                         u[:, bass.ts(mblk, MM)],
                                         start=step[s] == 0,
                                         stop=step[s] == nsteps - 1)
                        step[s] += 1
                # previous sample's PSUM->SBUF copies go after this chunk's
                # VectorE work so VectorE never stalls on the PE close
                if pending_copies:
                    for dst_sb, acc_ap in pending_copies:
                        nc.vector.tensor_copy(dst_sb, acc_ap)
                    pending_copies = []
                if s == 0 and c == len(CHUNKS) - 1:
                    accCD_, accA_ = accs[0]
                    pending_copies = [(out_sb[:, 0:MM], accCD_[:]),
                                      (out_sb[:, MM:2 * MM], accA_[:])]

            # last sample's PSUM->SBUF copies on the (by now idle) ScalarE
            accCD1, accA1 = accs[S - 1]
            nc.scalar.copy(out_sb[:, 2 * MM:3 * MM], accCD1[:])
            nc.scalar.copy(out_sb[:, 3 * MM:4 * MM], accA1[:])
            # partition-reduce the per-partition stats columns with two
            # tiny f32 matmuls so the whole result leaves in ONE contiguous
            # single-partition DMA (a [128,k] f32 output DMA is 128 tiny
            # strided descriptors -- measured ~2-3us of endgame)
            nc.tensor.matmul(accST[:, 0:NCOLS[0]], onesf[:], stats0[:],
                             start=True, stop=True)
            nc.tensor.matmul(accST[:, NCOLS[0]:NCOLS[0] + NCOLS[1]],
                             onesf[:], stats1[:], start=True, stop=True)
            nc.vector.tensor_copy(out_sb[:, 4 * MM:4 * MM + NC01], accST[:])
            nc.sync.dma_start(out_d[:], out_sb[:])
    nc.compile()
    return nc


def _get_nc():
    if "nc" not in _STATE:
        _STATE["nc"] = _build()
    return _STATE["nc"]


def _host_topk_fallback(p, g, m):
    """Exact per-sample reference semantics in numpy (rare path)."""
    p = p.astype(np.float32)
    positive = g * m
    negative = (1.0 - g) * m
    pos_count = positive.sum(dtype=np.float64)
    neg_count = min(negative.sum(dtype=np.float64), pos_count * NEG_RATIO)
    log_p = np.maximum(np.log(p), -100.0)
    log_1mp = np.maximum(np.log1p(-p), -100.0)
    loss = -(g * log_p + (1.0 - g) * log_1mp)
    pos_loss_sum = (loss * positive).sum(dtype=np.float64)
    neg_loss = (loss * negative).ravel()
    k = int(neg_count)
    if k > 0:
        top = np.partition(neg_loss, len(neg_loss) - k)[len(neg_loss) - k:]
        neg_topk = top.sum(dtype=np.float64)
    else:
        neg_topk = 0.0
    return (pos_loss_sum + neg_topk) / (pos_count + neg_count + EPS)


def _combine(results, p, g, m):
    losses = []
    for c in range(N_CORES):
        out = results[c]["out"].astype(np.float64).ravel()  # [OUTW]
        stc = out[4 * MM:]
        for s in range(S):
            nch = len(CHUNK_PLANS[s])
            base = 0 if s == 0 else NCOLS[0]
            M = stc[base:base + nch].sum()
            CD = out[2 * MM * s:2 * MM * s + MM].sum()
            A = out[2 * MM * s + MM:2 * MM * s + 2 * MM].sum()
            pos_count = round(A)
            neg_raw = round(M - A)
            if neg_raw <= pos_count * NEG_RATIO:
                # top-k covers every (strictly positive) negative loss
                losses.append((-CD) / (pos_count + neg_raw + EPS))
            else:
                i = c * S + s
                losses.append(_host_topk_fallback(p[i], g[i], m[i]))
    return np.float32(np.mean(losses))


def _in_maps(p, g, m):
    return [
        {"pred": p[c * S:(c + 1) * S],
         "gt": g[c * S:(c + 1) * S],
         "mask": m[c * S:(c + 1) * S]}
        for c in range(N_CORES)
    ]


def kernel(pred, gt, mask):
    from concourse import bass_utils

    p = np.ascontiguousarray(pred[:, 0], dtype=np.float32)   # [N,H,W]
    g = np.ascontiguousarray(gt, dtype=np.float32)
    m = np.ascontiguousarray(mask, dtype=np.float32)

    nc = _get_nc()
    in_maps = _in_maps(p, g, m)
    try:
        res = bass_utils.run_bass_kernel_spmd(nc, in_maps,
                                              core_ids=list(range(N_CORES)))
    except Exception:
        # one retry: transient device wedge from a prior process
        res = bass_utils.run_bass_kernel_spmd(nc, in_maps,
                                              core_ids=list(range(N_CORES)))
    return _combine(res.results, p, g, m)
